# revision 1
# baseline (speedup 1.0000x reference)
"""Trainium2 Bass kernel for the SIR-MLP network.

Computes, for each of B=65536 scenarios:
  gamma, beta, I0 = three tiny MLPs (16->10->10->10->10->1, tanh, softplus)
  then integrates the SIR ODE with RK4 over T=200 time points and returns
  the infected compartment at every time point, shape (T*B, 1) float32.

Strategy: pure data parallel over 8 NeuronCores (8192 scenarios each).
Inside a core the ODE state is tracked as u = (beta/N)*S and I, laid out
as [128 partitions x 64 free] tiles (sample s = 64*p + f). R is dropped
(it never feeds back and only I is output).

The packed state X_t = (I_t | u_t) lives inside the history buffer and
marches one 64-column slot to the right each step, so the I history
accumulates in place with zero copy instructions. All ODE arithmetic is
on the vector engine; per stage one packed [128,128] multiply computes
both derivative halves D = (r*I | -a*P) and one scalar_tensor_tensor
applies the RK4 update from the base state.
"""

import os
import sys

import numpy as np

try:
    import concourse.bass as bass  # noqa: F401
except ImportError:
    for _p in ("/opt/trn_rl_repo", os.path.expanduser("~/.axon_site/_ro/trn_rl_repo")):
        if os.path.isdir(_p) and _p not in sys.path:
            sys.path.insert(0, _p)

import concourse.bass as bass
import concourse.bacc as bacc
import concourse.mybir as mybir
import concourse.tile as tile
from concourse.bass_utils import run_bass_kernel_spmd

F32 = mybir.dt.float32
F32R = mybir.dt.float32r
AF = mybir.ActivationFunctionType
OP = mybir.AluOpType

B = 65536
IN = 16
HL = 10
NL = 3
T = 200
NPOP = 8.6e6
NCORES = 8
BL = B // NCORES          # 8192 samples per core
P = 128                   # partitions
FW = BL // P              # 64 free columns per state tile
CHUNK = 16                # time steps per history chunk tile
MMN = 512                 # matmul moving chunk
NB = 2                    # chunks banded together per ACT op (bases 0, 64)
USE_F32R = False

_cache = {}


def _build_program(dts):
    """Build the SPMD Bass program (one core's view). dts: list of floats."""
    nsteps = len(dts)
    nt = nsteps + 1  # number of output time points

    nc = bacc.Bacc("TRN2", target_bir_lowering=False, debug=False)

    xT = nc.declare_dram_parameter("xT", [IN, BL], F32, isOutput=False)
    w0p = nc.declare_dram_parameter("w0p", [IN, 3 * HL], F32, isOutput=False)
    whp = nc.declare_dram_parameter("whp", [NL, 94, 3 * HL], F32, isOutput=False)
    wop = nc.declare_dram_parameter("wop", [94, 3], F32, isOutput=False)
    b0c = nc.declare_dram_parameter("b0c", [94, 1], F32, isOutput=False)
    bhc = nc.declare_dram_parameter("bhc", [NL, 94, 1], F32, isOutput=False)
    boc = nc.declare_dram_parameter("boc", [67, 1], F32, isOutput=False)
    out = nc.declare_dram_parameter("out", [nt, BL], F32, isOutput=True)

    spbuf = nc.dram_tensor("spbuf", [3, BL], F32)  # softplus outputs bounce

    def mmdt(ap):
        return ap.bitcast(F32R) if USE_F32R else ap

    # History chunk layout: chunk k holds I_t slots for t in [16k, 16k+16)
    # plus one extra slot: the packed state X_t=(I_t|u_t) spans slots
    # [j, j+1] so the last in-chunk step reads its u from the extra slot.
    # Final chunk: last_slots I-slots + 1 slot for the dead final u.
    nfull = nt // CHUNK if nt % CHUNK else nt // CHUNK - 1
    chunk_cols = [(CHUNK + 1) * FW] * nfull
    last_slots = nt - nfull * CHUNK
    chunk_cols.append((last_slots + 1) * FW)
    nchunks = len(chunk_cols)

    with tile.TileContext(nc) as tc:
        with (
            tc.tile_pool(name="const", bufs=1) as cpool,
            tc.tile_pool(name="hist", bufs=1) as hpool,
            tc.tile_pool(name="mlp", bufs=3) as mpool,
            tc.tile_pool(name="psum", bufs=1, space="PSUM") as ppool,
            tc.tile_pool(name="work", bufs=1) as wpool,
        ):
            # ---------------- MLP phase ----------------
            xt_s = cpool.tile([IN, BL], F32, tag="xt")
            nc.sync.dma_start(xt_s[:], xT[:])
            w0_s = cpool.tile([IN, 3 * HL], F32, tag="w0")
            nc.sync.dma_start(w0_s[:], w0p[:])
            wh_s = []
            bh_s = []
            for l in range(NL):
                w = cpool.tile([94, 3 * HL], F32, tag=f"wh{l}")
                nc.sync.dma_start(w[:], whp[l])
                wh_s.append(w)
                bb = cpool.tile([94, 1], F32, tag=f"bh{l}")
                nc.sync.dma_start(bb[:], bhc[l])
                bh_s.append(bb)
            wo_s = cpool.tile([94, 3], F32, tag="wo")
            nc.sync.dma_start(wo_s[:], wop[:])
            b0_s = cpool.tile([94, 1], F32, tag="b0")
            nc.sync.dma_start(b0_s[:], b0c[:])
            bo_s = cpool.tile([67, 1], F32, tag="bo")
            nc.sync.dma_start(bo_s[:], boc[:])

            # fp32 Matmult lowers with a self-loading-weights struct with
            # tight sync-wait limits; sync each stationary tensor to PE via
            # a tiny dummy matmul (also warms up the PE pipeline).
            dummy_ps = ppool.tile([3 * HL, 1], F32, tag="dummy")
            nc.tensor.matmul(dummy_ps[:], w0_s[:], w0_s[:, :1],
                             start=True, stop=True)
            for l in range(NL):
                nc.tensor.matmul(dummy_ps[:], wh_s[l][0:30, :],
                                 wh_s[l][0:30, :1], start=True, stop=True)
            nc.tensor.matmul(dummy_ps[:3, :], wo_s[0:30, :], wo_s[0:30, :1],
                             start=True, stop=True)

            # Layer-by-layer across groups: PE overlaps ACT, activation
            # tables load once per function, PE stays ramped.
            ngroup = BL // (MMN * NB)
            hcur = [None] * ngroup
            for grp in range(ngroup):
                ph = ppool.tile([94, MMN], F32, tag="ph", bufs=3)
                for b_ in range(NB):
                    lo = (grp * NB + b_) * MMN
                    nc.tensor.matmul(ph[64 * b_ : 64 * b_ + 30, :], mmdt(w0_s[:]),
                                     mmdt(xt_s[:, lo : lo + MMN]),
                                     start=True, stop=True)
                h = mpool.tile([94, MMN], F32, tag="h", bufs=12, name="h")
                nc.scalar.activation(h[:], ph[:], AF.Tanh, bias=b0_s[:])
                hcur[grp] = h
            for l in range(NL):
                for grp in range(ngroup):
                    ph2 = ppool.tile([94, MMN], F32, tag="ph", bufs=3)
                    for b_ in range(NB):
                        nc.tensor.matmul(ph2[64 * b_ : 64 * b_ + 30, :],
                                         mmdt(wh_s[l][64 * b_ : 64 * b_ + 30, :]),
                                         mmdt(hcur[grp][64 * b_ : 64 * b_ + 30, :]),
                                         start=True, stop=True)
                    h = mpool.tile([94, MMN], F32, tag="h", bufs=12, name="h")
                    nc.scalar.activation(h[:], ph2[:], AF.Tanh, bias=bh_s[l][:])
                    hcur[grp] = h
            ecur = [None] * ngroup
            for grp in range(ngroup):
                po = ppool.tile([67, MMN], F32, tag="po", bufs=3)
                for b_ in range(NB):
                    nc.tensor.matmul(po[64 * b_ : 64 * b_ + 3, :],
                                     mmdt(wo_s[64 * b_ : 64 * b_ + 30, :]),
                                     mmdt(hcur[grp][64 * b_ : 64 * b_ + 30, :]),
                                     start=True, stop=True)
                e = mpool.tile([67, MMN], F32, tag="e", bufs=8, name="e")
                nc.scalar.activation(e[:], po[:], AF.Exp, bias=bo_s[:])
                ecur[grp] = e
            for grp in range(ngroup):
                sp = mpool.tile([67, MMN], F32, tag="sp", bufs=3, name="sp")
                # softplus = ln(1 + exp(x))
                nc.scalar.activation(sp[:], ecur[grp][:], AF.Ln, bias=1.0)
                for b_ in range(NB):
                    lo = (grp * NB + b_) * MMN
                    nc.sync.dma_start(spbuf[:, lo : lo + MMN],
                                      sp[64 * b_ : 64 * b_ + 3, :])

            # ---------------- constants ----------------
            gam = cpool.tile([P, FW], F32, tag="gam")
            nc.sync.dma_start(gam[:], spbuf[0].rearrange("(p f) -> p f", f=FW))
            bet = cpool.tile([P, FW], F32, tag="bet")
            nc.sync.dma_start(bet[:], spbuf[1].rearrange("(p f) -> p f", f=FW))

            hist = [hpool.tile([P, cols], F32, tag=f"hc{k}", name=f"hc{k}")
                    for k, cols in enumerate(chunk_cols)]
            # I_0 goes straight into history chunk 0, slot 0
            nc.sync.dma_start(hist[0][:, 0:FW],
                              spbuf[2].rearrange("(p f) -> p f", f=FW))

            # M = (r | na): na = -beta/NPOP; r refilled every RK stage
            m1 = cpool.tile([P, 2 * FW], F32, tag="m1")
            na = m1[:, FW:]
            nc.vector.tensor_scalar_mul(na, bet[:], -1.0 / NPOP)

            # u_0 = a*(NPOP - I_0) = na*(I_0 - NPOP), into X_0's u slot
            s0n = cpool.tile([P, FW], F32, tag="s0n")
            nc.vector.tensor_scalar(s0n[:], hist[0][:, 0:FW], 1.0, -NPOP,
                                    OP.mult, OP.add)
            nc.vector.tensor_tensor(hist[0][:, FW : 2 * FW], na, s0n[:], OP.mult)

            # scratch W = (I_j | P_j/u_j) and D tiles
            w_s = wpool.tile([P, 2 * FW], F32, tag="W")
            d_t = [wpool.tile([P, 2 * FW], F32, tag=f"D{i}", name=f"D{i}")
                   for i in range(4)]

            vec = nc.vector
            # ---------------- RK4 time stepping ----------------
            for t in range(nsteps):
                h_dt = float(dts[t])
                c1 = 0.5 * h_dt
                w6 = h_dt / 6.0

                k, j = divmod(t, CHUNK)
                ck = hist[k]
                x0 = ck[:, j * FW : (j + 2) * FW]        # (I_t | u_t)
                i_t = ck[:, j * FW : (j + 1) * FW]
                u_t = ck[:, (j + 1) * FW : (j + 2) * FW]
                if j < CHUNK - 1 or k == nchunks - 1:
                    x_dst = ck[:, (j + 1) * FW : (j + 3) * FW]
                else:
                    x_dst = hist[k + 1][:, 0 : 2 * FW]

                d1, d2, d3, d4 = d_t
                wi = w_s[:, 0:FW]        # I_j slot
                wu = w_s[:, FW : 2 * FW]  # P_j / u_j slot
                wv = w_s[:, 0 : 2 * FW]   # (I_j | P_j)
                r_ = m1[:, 0:FW]

                # stage 1 (reads X_t in place from history; split D ops)
                vec.tensor_tensor(r_, u_t, gam[:], OP.subtract)
                vec.tensor_tensor(d1[:, 0:FW], r_, i_t, OP.mult)       # K1
                vec.tensor_tensor(wu, u_t, i_t, OP.mult)               # P1
                vec.tensor_tensor(d1[:, FW:], na, wu, OP.mult)         # -Q1
                vec.scalar_tensor_tensor(w_s[:], d1[:], c1, x0,
                                         OP.mult, OP.add)              # X2

                # stages 2 and 3
                for d_, cc in ((d2, c1), (d3, h_dt)):
                    vec.tensor_tensor(r_, wu, gam[:], OP.subtract)
                    vec.tensor_tensor(wu, wu, wi, OP.mult)     # P in place of u
                    vec.tensor_tensor(d_[:], m1[:], wv, OP.mult)
                    vec.scalar_tensor_tensor(w_s[:], d_[:], cc, x0,
                                             OP.mult, OP.add)

                # stage 4
                vec.tensor_tensor(r_, wu, gam[:], OP.subtract)
                vec.tensor_tensor(wu, wu, wi, OP.mult)
                vec.tensor_tensor(d4[:], m1[:], wv, OP.mult)

                # combine: X_new = X0 + h/6 * ((D1+D4) + 2*(D2+D3))
                vec.tensor_tensor(d1[:], d1[:], d4[:], OP.add)
                vec.tensor_tensor(d2[:], d2[:], d3[:], OP.add)
                vec.scalar_tensor_tensor(d2[:], d2[:], 2.0, d1[:],
                                         OP.mult, OP.add)
                vec.scalar_tensor_tensor(x_dst, d2[:], w6, x0,
                                         OP.mult, OP.add)

                # chunk complete -> DMA out its 16 I slots
                if j == CHUNK - 1 or t == nsteps - 1:
                    nslots = CHUNK if j == CHUNK - 1 else last_slots
                    t0 = k * CHUNK
                    src = ck[:, 0 : nslots * FW].rearrange(
                        "p (t f) -> p t f", f=FW)
                    dst = out[t0 : t0 + nslots, :].rearrange(
                        "t (p f) -> p t f", p=P)
                    nc.sync.dma_start(dst, src)

    nc.compile()
    return nc


def _pack_params(W0, b0, Wh, bh, Wo, bo):
    W0p = np.ascontiguousarray(W0.transpose(2, 0, 1).reshape(IN, 3 * HL))
    b0c = np.zeros((94, 1), np.float32)
    boc = np.zeros((67, 1), np.float32)
    bhc = np.zeros((NL, 94, 1), np.float32)
    whs = np.zeros((3 * HL, 3 * HL), np.float32)
    Whp = np.zeros((NL, 94, 3 * HL), np.float32)
    for l in range(NL):
        whs[:] = 0
        for n in range(3):
            whs[n * HL : (n + 1) * HL, n * HL : (n + 1) * HL] = Wh[n, l].T
        Whp[l, 0:30] = whs
        Whp[l, 64:94] = whs
    wos = np.zeros((3 * HL, 3), np.float32)
    for n in range(3):
        wos[n * HL : (n + 1) * HL, n] = Wo[n, 0]
    Wop = np.zeros((94, 3), np.float32)
    Wop[0:30] = wos
    Wop[64:94] = wos
    for b_ in range(NB):
        b0c[64 * b_ : 64 * b_ + 30] = b0.reshape(3 * HL, 1)
        boc[64 * b_ : 64 * b_ + 3] = bo.reshape(3, 1)
        for l in range(NL):
            bhc[l, 64 * b_ : 64 * b_ + 30] = bh[:, l].reshape(3 * HL, 1)
    return (np.ascontiguousarray(W0p), np.ascontiguousarray(b0c),
            Whp, bhc, Wop, np.ascontiguousarray(boc))


def _make_in_maps(data, W0, b0, Wh, bh, Wo, bo):
    W0p, b0c, Whp, bhc, Wop, boc = _pack_params(
        np.asarray(W0, np.float32), np.asarray(b0, np.float32),
        np.asarray(Wh, np.float32), np.asarray(bh, np.float32),
        np.asarray(Wo, np.float32), np.asarray(bo, np.float32))
    dataT = np.ascontiguousarray(np.asarray(data, np.float32).T)  # [16, B]
    shared = {"w0p": W0p, "whp": Whp, "wop": Wop,
              "b0c": b0c, "bhc": bhc, "boc": boc}
    in_maps = []
    for c in range(NCORES):
        m = dict(shared)
        m["xT"] = np.ascontiguousarray(dataT[:, c * BL : (c + 1) * BL])
        in_maps.append(m)
    return in_maps


def _get_program(times):
    dts = np.diff(np.asarray(times, np.float64)).astype(np.float32)
    key = dts.tobytes()
    if key not in _cache:
        _cache[key] = _build_program([float(x) for x in dts])
    return _cache[key]


def kernel(data, times, W0, b0, Wh, bh, Wo, bo):
    nc = _get_program(times)
    in_maps = _make_in_maps(data, W0, b0, Wh, bh, Wo, bo)
    res = run_bass_kernel_spmd(nc, in_maps, list(range(NCORES)))

    nt = len(times)
    full = np.empty((nt, B), np.float32)
    for c in range(NCORES):
        full[:, c * BL : (c + 1) * BL] = res.results[c]["out"]
    return full.reshape(nt * B, 1)


def timed_run(inputs):
    """Run once with NTFF tracing enabled; returns exec_time_ns (or None)."""
    nc = _get_program(np.asarray(inputs["times"], np.float32))
    in_maps = _make_in_maps(inputs["data"], inputs["W0"], inputs["b0"],
                            inputs["Wh"], inputs["bh"], inputs["Wo"],
                            inputs["bo"])
    import shutil
    tdir = "/root/problem/trace_out"
    shutil.rmtree(tdir, ignore_errors=True)
    os.makedirs(tdir, exist_ok=True)
    res = run_bass_kernel_spmd(nc, in_maps, list(range(NCORES)), trace=True,
                               tmpdir=tdir)
    return res.exec_time_ns



# revision 3
# speedup vs baseline: 1.3086x; 1.3086x over previous
"""Trainium2 Bass kernel for the SIR-MLP network.

Computes, for each of B=65536 scenarios:
  gamma, beta, I0 = three tiny MLPs (16->10->10->10->10->1, tanh, softplus)
  then integrates the SIR ODE with RK4 over T=200 time points and returns
  the infected compartment at every time point, shape (T*B, 1) float32.

Strategy: pure data parallel over 8 NeuronCores (8192 scenarios each).
Inside a core the ODE state is tracked as u = (beta/N)*S and I, laid out
as [128 partitions x 64 free] (sample s = 64*p + f). The 64 free columns
are split into independent engine streams: the DVE (vector) engine owns
most columns and the GpSimd (Pool) engine owns the rest, each running its
own RK4 chain with zero cross-engine dependencies inside the loop.

Per stream the packed state X_t = (I_t | u_t) marches through history
chunk tiles (accumulating the I output in place), and the RK4 combine is
restructured as an incremental accumulator A = X0 + h/6 D1 + h/3 D2 +
h/3 D3 (off the critical chain), so the dependent chain per step is only
~12 ops with independent accumulator work filling the RAW-interlock
stalls.
"""

import os
import sys

import numpy as np

try:
    import concourse.bass as bass  # noqa: F401
except ImportError:
    for _p in ("/opt/trn_rl_repo", os.path.expanduser("~/.axon_site/_ro/trn_rl_repo")):
        if os.path.isdir(_p) and _p not in sys.path:
            sys.path.insert(0, _p)

import concourse.bass as bass
import concourse.bacc as bacc
import concourse.mybir as mybir
import concourse.tile as tile
from concourse.bass_utils import run_bass_kernel_spmd

F32 = mybir.dt.float32
F32R = mybir.dt.float32r
AF = mybir.ActivationFunctionType
OP = mybir.AluOpType

B = 65536
IN = 16
HL = 10
NL = 3
T = 200
NPOP = 8.6e6
NCORES = 8
BL = B // NCORES          # 8192 samples per core
P = 128                   # partitions
FW = BL // P              # 64 free columns of state per core
CHUNK = 16                # time steps per history chunk tile
MMN = 512                 # matmul moving chunk
NB = 2                    # chunks banded together per ACT op (bases 0, 64)
USE_F32R = False

POOL_COLS = 0             # free columns owned by the GpSimd stream
DVE_STREAMS = 1           # independent column streams on the vector engine

_cache = {}


def _stream_plan():
    """Return [(engine_name, col_start, col_width), ...] covering FW cols."""
    plan = []
    dcols = FW - POOL_COLS
    base = 0
    for s in range(DVE_STREAMS):
        w = dcols // DVE_STREAMS + (1 if s < dcols % DVE_STREAMS else 0)
        plan.append(("vector", base, w))
        base += w
    if POOL_COLS:
        plan.append(("gpsimd", base, POOL_COLS))
    return plan


class _Stream:
    """Per-engine-stream state for the RK4 loop."""

    def __init__(self, nc, pool, eng_name, c0, w, chunk_cols, gam, bet):
        self.eng = getattr(nc, eng_name)
        self.c0 = c0
        self.w = w
        self.gam = gam[:, c0 : c0 + w]
        self.bet = bet[:, c0 : c0 + w]
        tagp = f"{eng_name}{c0}"
        self.hist = [
            pool.tile([P, cols], F32, tag=f"h{tagp}_{k}", name=f"h{tagp}_{k}")
            for k, cols in enumerate(chunk_cols)
        ]
        # M = (r | na): na = -beta/NPOP constant; r refilled every RK stage
        self.m1 = pool.tile([P, 2 * w], F32, tag=f"m{tagp}")
        self.w_s = pool.tile([P, 2 * w], F32, tag=f"W{tagp}")
        self.dd = pool.tile([P, 2 * w], F32, tag=f"D{tagp}")
        self.acc = pool.tile([P, 2 * w], F32, tag=f"A{tagp}")


def _emit_step(st, t, dts, chunk_cols):
    """Emit one RK4 step for stream st at time index t."""
    h_dt = float(dts[t])
    c1 = 0.5 * h_dt
    w6 = h_dt / 6.0
    w3 = h_dt / 3.0
    w = st.w
    eng = st.eng
    nchunks = len(chunk_cols)

    k, j = divmod(t, CHUNK)
    ck = st.hist[k]
    x0 = ck[:, j * w : (j + 2) * w]          # (I_t | u_t)
    i_t = ck[:, j * w : (j + 1) * w]
    u_t = ck[:, (j + 1) * w : (j + 2) * w]
    if j < CHUNK - 1 or k == nchunks - 1:
        x_dst = ck[:, (j + 1) * w : (j + 3) * w]
    else:
        x_dst = st.hist[k + 1][:, 0 : 2 * w]

    dd = st.dd
    wi = st.w_s[:, 0:w]
    wu = st.w_s[:, w : 2 * w]
    wv = st.w_s[:, 0 : 2 * w]
    r_ = st.m1[:, 0:w]
    na = st.m1[:, w : 2 * w]
    A = st.acc

    # stage 1: D1 = ((u-g)*I | na*u*I), reading X_t in place from history
    eng.tensor_tensor(r_, u_t, st.gam, OP.subtract)
    eng.tensor_tensor(wu, u_t, i_t, OP.mult)                  # P1
    eng.tensor_tensor(dd[:, 0:w], r_, i_t, OP.mult)
    eng.tensor_tensor(dd[:, w:], na, wu, OP.mult)
    eng.scalar_tensor_tensor(st.w_s[:], dd[:], c1, x0, OP.mult, OP.add)  # X2
    eng.scalar_tensor_tensor(A[:], dd[:], w6, x0, OP.mult, OP.add)  # off-chain

    # stages 2 and 3: X_{s+1} = X0 + c*Ds, A += h/3 * Ds
    for cc in (c1, h_dt):
        eng.tensor_tensor(r_, wu, st.gam, OP.subtract)
        eng.tensor_tensor(wu, wu, wi, OP.mult)                # P in place
        eng.tensor_tensor(dd[:], st.m1[:], wv, OP.mult)
        eng.scalar_tensor_tensor(st.w_s[:], dd[:], cc, x0, OP.mult, OP.add)
        eng.scalar_tensor_tensor(A[:], dd[:], w3, A[:], OP.mult, OP.add)

    # stage 4: X_new = A + h/6 * D4
    eng.tensor_tensor(r_, wu, st.gam, OP.subtract)
    eng.tensor_tensor(wu, wu, wi, OP.mult)
    eng.tensor_tensor(dd[:], st.m1[:], wv, OP.mult)
    eng.scalar_tensor_tensor(x_dst, dd[:], w6, A[:], OP.mult, OP.add)


def _build_program(dts):
    """Build the SPMD Bass program (one core's view). dts: list of floats."""
    nsteps = len(dts)
    nt = nsteps + 1  # number of output time points

    nc = bacc.Bacc("TRN2", target_bir_lowering=False, debug=False)

    xT = nc.declare_dram_parameter("xT", [IN, BL], F32, isOutput=False)
    w0p = nc.declare_dram_parameter("w0p", [IN, 3 * HL], F32, isOutput=False)
    whp = nc.declare_dram_parameter("whp", [NL, 94, 3 * HL], F32, isOutput=False)
    wop = nc.declare_dram_parameter("wop", [94, 3], F32, isOutput=False)
    b0c = nc.declare_dram_parameter("b0c", [94, 1], F32, isOutput=False)
    bhc = nc.declare_dram_parameter("bhc", [NL, 94, 1], F32, isOutput=False)
    boc = nc.declare_dram_parameter("boc", [67, 1], F32, isOutput=False)
    out = nc.declare_dram_parameter("out", [nt, BL], F32, isOutput=True)

    spbuf = nc.dram_tensor("spbuf", [3, BL], F32)  # softplus outputs bounce

    def mmdt(ap):
        return ap.bitcast(F32R) if USE_F32R else ap

    # History chunk layout (per stream of width w): chunk k holds I_t slots
    # for t in [16k, 16k+16) plus one extra slot: the packed state
    # X_t=(I_t|u_t) spans slots [j, j+1] so the last in-chunk step reads its
    # u from the extra slot. Final chunk: last_slots I-slots + 1 for the
    # dead final u. chunk_cols counts SLOTS here; each stream scales by w.
    nfull = nt // CHUNK if nt % CHUNK else nt // CHUNK - 1
    chunk_slots = [CHUNK + 1] * nfull
    last_slots = nt - nfull * CHUNK
    chunk_slots.append(last_slots + 1)

    plan = _stream_plan()

    with tile.TileContext(nc) as tc:
        with (
            tc.tile_pool(name="const", bufs=1) as cpool,
            tc.tile_pool(name="hist", bufs=1) as hpool,
            tc.tile_pool(name="mlp", bufs=3) as mpool,
            tc.tile_pool(name="psum", bufs=1, space="PSUM") as ppool,
        ):
            # ---------------- MLP phase ----------------
            xt_s = cpool.tile([IN, BL], F32, tag="xt")
            nc.sync.dma_start(xt_s[:], xT[:])
            w0_s = cpool.tile([IN, 3 * HL], F32, tag="w0")
            nc.sync.dma_start(w0_s[:], w0p[:])
            wh_s = []
            bh_s = []
            for l in range(NL):
                w = cpool.tile([94, 3 * HL], F32, tag=f"wh{l}")
                nc.sync.dma_start(w[:], whp[l])
                wh_s.append(w)
                bb = cpool.tile([94, 1], F32, tag=f"bh{l}")
                nc.sync.dma_start(bb[:], bhc[l])
                bh_s.append(bb)
            wo_s = cpool.tile([94, 3], F32, tag="wo")
            nc.sync.dma_start(wo_s[:], wop[:])
            b0_s = cpool.tile([94, 1], F32, tag="b0")
            nc.sync.dma_start(b0_s[:], b0c[:])
            bo_s = cpool.tile([67, 1], F32, tag="bo")
            nc.sync.dma_start(bo_s[:], boc[:])

            # fp32 Matmult lowers with a self-loading-weights struct with
            # tight sync-wait limits; sync each stationary tensor to PE via
            # a tiny dummy matmul (also warms up the PE pipeline).
            dummy_ps = ppool.tile([3 * HL, 1], F32, tag="dummy")
            nc.tensor.matmul(dummy_ps[:], w0_s[:], w0_s[:, :1],
                             start=True, stop=True)
            for l in range(NL):
                nc.tensor.matmul(dummy_ps[:], wh_s[l][0:30, :],
                                 wh_s[l][0:30, :1], start=True, stop=True)
            nc.tensor.matmul(dummy_ps[:3, :], wo_s[0:30, :], wo_s[0:30, :1],
                             start=True, stop=True)

            # Layer-by-layer across groups: PE overlaps ACT, activation
            # tables load once per function, PE stays ramped.
            ngroup = BL // (MMN * NB)
            hcur = [None] * ngroup
            for grp in range(ngroup):
                ph = ppool.tile([94, MMN], F32, tag="ph", bufs=3)
                for b_ in range(NB):
                    lo = (grp * NB + b_) * MMN
                    nc.tensor.matmul(ph[64 * b_ : 64 * b_ + 30, :], mmdt(w0_s[:]),
                                     mmdt(xt_s[:, lo : lo + MMN]),
                                     start=True, stop=True)
                h = mpool.tile([94, MMN], F32, tag="h", bufs=12, name="h")
                nc.scalar.activation(h[:], ph[:], AF.Tanh, bias=b0_s[:])
                hcur[grp] = h
            for l in range(NL):
                for grp in range(ngroup):
                    ph2 = ppool.tile([94, MMN], F32, tag="ph", bufs=3)
                    for b_ in range(NB):
                        nc.tensor.matmul(ph2[64 * b_ : 64 * b_ + 30, :],
                                         mmdt(wh_s[l][64 * b_ : 64 * b_ + 30, :]),
                                         mmdt(hcur[grp][64 * b_ : 64 * b_ + 30, :]),
                                         start=True, stop=True)
                    h = mpool.tile([94, MMN], F32, tag="h", bufs=12, name="h")
                    nc.scalar.activation(h[:], ph2[:], AF.Tanh, bias=bh_s[l][:])
                    hcur[grp] = h
            ecur = [None] * ngroup
            for grp in range(ngroup):
                po = ppool.tile([67, MMN], F32, tag="po", bufs=3)
                for b_ in range(NB):
                    nc.tensor.matmul(po[64 * b_ : 64 * b_ + 3, :],
                                     mmdt(wo_s[64 * b_ : 64 * b_ + 30, :]),
                                     mmdt(hcur[grp][64 * b_ : 64 * b_ + 30, :]),
                                     start=True, stop=True)
                e = mpool.tile([67, MMN], F32, tag="e", bufs=8, name="e")
                nc.scalar.activation(e[:], po[:], AF.Exp, bias=bo_s[:])
                ecur[grp] = e
            for grp in range(ngroup):
                sp = mpool.tile([67, MMN], F32, tag="sp", bufs=3, name="sp")
                # softplus = ln(1 + exp(x))
                nc.scalar.activation(sp[:], ecur[grp][:], AF.Ln, bias=1.0)
                for b_ in range(NB):
                    lo = (grp * NB + b_) * MMN
                    nc.sync.dma_start(spbuf[:, lo : lo + MMN],
                                      sp[64 * b_ : 64 * b_ + 3, :])

            # ---------------- constants ----------------
            gam = cpool.tile([P, FW], F32, tag="gam")
            nc.sync.dma_start(gam[:], spbuf[0].rearrange("(p f) -> p f", f=FW))
            bet = cpool.tile([P, FW], F32, tag="bet")
            nc.sync.dma_start(bet[:], spbuf[1].rearrange("(p f) -> p f", f=FW))

            streams = [
                _Stream(nc, hpool, en, c0, w,
                        [s * w for s in chunk_slots], gam, bet)
                for (en, c0, w) in plan
            ]

            i0_src = spbuf[2].rearrange("(p f) -> p f", f=FW)
            s0n = cpool.tile([P, FW], F32, tag="s0n")
            for st in streams:
                c0, w = st.c0, st.w
                # I_0 straight into history chunk 0, slot 0
                nc.sync.dma_start(st.hist[0][:, 0:w], i0_src[:, c0 : c0 + w])
                na = st.m1[:, w : 2 * w]
                nc.vector.tensor_scalar_mul(na, st.bet, -1.0 / NPOP)
                # u_0 = a*(NPOP - I_0) = na*(I_0 - NPOP), into X_0's u slot
                nc.vector.tensor_scalar(s0n[:, c0 : c0 + w],
                                        st.hist[0][:, 0:w], 1.0, -NPOP,
                                        OP.mult, OP.add)
                nc.vector.tensor_tensor(st.hist[0][:, w : 2 * w], na,
                                        s0n[:, c0 : c0 + w], OP.mult)

            # ---------------- RK4 time stepping ----------------
            for t in range(nsteps):
                for st in streams:
                    _emit_step(st, t, dts, chunk_slots)

                # chunk complete -> DMA out its 16 I slots per stream
                k, j = divmod(t, CHUNK)
                if j == CHUNK - 1 or t == nsteps - 1:
                    nslots = CHUNK if j == CHUNK - 1 else last_slots
                    t0 = k * CHUNK
                    full_dst = out[t0 : t0 + nslots, :].rearrange(
                        "t (p f) -> p t f", p=P)
                    for st in streams:
                        src = st.hist[k][:, 0 : nslots * st.w].rearrange(
                            "p (t f) -> p t f", f=st.w)
                        nc.sync.dma_start(
                            full_dst[:, :, st.c0 : st.c0 + st.w], src)

    nc.compile()
    return nc


def _pack_params(W0, b0, Wh, bh, Wo, bo):
    W0p = np.ascontiguousarray(W0.transpose(2, 0, 1).reshape(IN, 3 * HL))
    b0c = np.zeros((94, 1), np.float32)
    boc = np.zeros((67, 1), np.float32)
    bhc = np.zeros((NL, 94, 1), np.float32)
    whs = np.zeros((3 * HL, 3 * HL), np.float32)
    Whp = np.zeros((NL, 94, 3 * HL), np.float32)
    for l in range(NL):
        whs[:] = 0
        for n in range(3):
            whs[n * HL : (n + 1) * HL, n * HL : (n + 1) * HL] = Wh[n, l].T
        Whp[l, 0:30] = whs
        Whp[l, 64:94] = whs
    wos = np.zeros((3 * HL, 3), np.float32)
    for n in range(3):
        wos[n * HL : (n + 1) * HL, n] = Wo[n, 0]
    Wop = np.zeros((94, 3), np.float32)
    Wop[0:30] = wos
    Wop[64:94] = wos
    for b_ in range(NB):
        b0c[64 * b_ : 64 * b_ + 30] = b0.reshape(3 * HL, 1)
        boc[64 * b_ : 64 * b_ + 3] = bo.reshape(3, 1)
        for l in range(NL):
            bhc[l, 64 * b_ : 64 * b_ + 30] = bh[:, l].reshape(3 * HL, 1)
    return (np.ascontiguousarray(W0p), np.ascontiguousarray(b0c),
            Whp, bhc, Wop, np.ascontiguousarray(boc))


def _make_in_maps(data, W0, b0, Wh, bh, Wo, bo):
    W0p, b0c, Whp, bhc, Wop, boc = _pack_params(
        np.asarray(W0, np.float32), np.asarray(b0, np.float32),
        np.asarray(Wh, np.float32), np.asarray(bh, np.float32),
        np.asarray(Wo, np.float32), np.asarray(bo, np.float32))
    dataT = np.ascontiguousarray(np.asarray(data, np.float32).T)  # [16, B]
    shared = {"w0p": W0p, "whp": Whp, "wop": Wop,
              "b0c": b0c, "bhc": bhc, "boc": boc}
    in_maps = []
    for c in range(NCORES):
        m = dict(shared)
        m["xT"] = np.ascontiguousarray(dataT[:, c * BL : (c + 1) * BL])
        in_maps.append(m)
    return in_maps


def _get_program(times):
    dts = np.diff(np.asarray(times, np.float64)).astype(np.float32)
    key = dts.tobytes()
    if key not in _cache:
        _cache[key] = _build_program([float(x) for x in dts])
    return _cache[key]


def kernel(data, times, W0, b0, Wh, bh, Wo, bo):
    nc = _get_program(times)
    in_maps = _make_in_maps(data, W0, b0, Wh, bh, Wo, bo)
    res = run_bass_kernel_spmd(nc, in_maps, list(range(NCORES)))

    nt = len(times)
    full = np.empty((nt, B), np.float32)
    for c in range(NCORES):
        full[:, c * BL : (c + 1) * BL] = res.results[c]["out"]
    return full.reshape(nt * B, 1)


def timed_run(inputs):
    """Run once with NTFF tracing enabled; returns exec_time_ns (or None)."""
    nc = _get_program(np.asarray(inputs["times"], np.float32))
    in_maps = _make_in_maps(inputs["data"], inputs["W0"], inputs["b0"],
                            inputs["Wh"], inputs["bh"], inputs["Wo"],
                            inputs["bo"])
    import shutil
    tdir = "/root/problem/trace_out"
    shutil.rmtree(tdir, ignore_errors=True)
    os.makedirs(tdir, exist_ok=True)
    res = run_bass_kernel_spmd(nc, in_maps, list(range(NCORES)), trace=True,
                               tmpdir=tdir)
    return res.exec_time_ns
